# revision 23
# baseline (speedup 1.0000x reference)
"""Bass/Trainium2 kernel for nn_CharLevelLanguageModel (6-layer char transformer).

Strategy: data-parallel over batch (64 -> 8 cores x 8). Full forward in one NEFF
per core, emitted as a software pipeline over 24 iterations (6 layers x 4
batch-pairs; each layer is fully local to a 512-token batch-pair). Emission
order per step is  A(i+1) | Wo+LN2(i) | ATT(i+1) | FFN(i)  so every serial
dependency chain (LN row ops, softmax normalize) is covered by another
iteration's matmul work and the PE stays busy (warm HAM clock).

Activations are feature-major f32r. LayerNorm affine params are folded into
adjacent weights on the host; on-device LN is standardization via ones-matmul
stats + row chain + gpsimd partition broadcasts. Attention is key-major:
softmax needs no transposes; sums ride a ones-column appended to V; causal
masking is one multiplicative gpsimd multiply per (head, batch).
"""

import os
import numpy as np

import concourse.bass as bass
import concourse.mybir as mybir
import concourse.tile as tile
from concourse import bacc
from concourse.bass_utils import run_bass_kernel_spmd

B, T, C, H, L, V = 64, 256, 384, 6, 6, 65
HS = C // H          # 64
DFF = 4 * C          # 1536
N_CORES = 8
BPC = B // N_CORES   # 8 batches per core
NTOK = BPC * T       # 2048 tokens per core
NBP = 4              # batch-pair (512-token) tiles per core
KC = C // 128        # 3 feature chunks
K12 = DFF // 128     # 12 dff chunks
EPS = 1e-5
SCALE = HS ** -0.5

f32 = mybir.dt.float32
f32r = mybir.dt.float32r
AF = mybir.ActivationFunctionType
ALU = mybir.AluOpType

N_LAYERS = int(os.environ.get("KERNEL_LAYERS", str(L)))

_cache = {}


def _build_nc():
    nc = bacc.Bacc("TRN2", target_bir_lowering=False, debug=False,
                   num_devices=N_CORES)

    x0T_d = nc.dram_tensor("x0T", [C, NTOK], f32r, kind="ExternalInput").ap()
    wqkv_d = nc.dram_tensor("wqkv", [L, C, 3 * C], f32r, kind="ExternalInput").ap()
    bqkv_d = nc.dram_tensor("bqkv", [L, 128, 6], f32, kind="ExternalInput").ap()
    wo_d = nc.dram_tensor("wo", [L, C, C], f32r, kind="ExternalInput").ap()
    w1_d = nc.dram_tensor("w1", [L, C, DFF], f32r, kind="ExternalInput").ap()
    b1_d = nc.dram_tensor("b1", [L, 128, K12], f32, kind="ExternalInput").ap()
    w2_d = nc.dram_tensor("w2", [L, DFF, C], f32r, kind="ExternalInput").ap()
    brows_d = nc.dram_tensor("brows", [L, 1, 2 * C], f32r, kind="ExternalInput").ap()
    wlm_d = nc.dram_tensor("wlm", [C, V], f32r, kind="ExternalInput").ap()
    blm_d = nc.dram_tensor("blm", [V], f32, kind="ExternalInput").ap()
    m01_d = nc.dram_tensor("m01", [128, 512], f32, kind="ExternalInput").ap()
    outT_d = nc.dram_tensor("outT", [V, NTOK], f32, kind="ExternalOutput").ap()

    with tile.TileContext(nc) as tc:
        _build_body(nc, tc, x0T_d, wqkv_d, bqkv_d, wo_d, w1_d, b1_d, w2_d,
                    brows_d, wlm_d, blm_d, m01_d, outT_d)
    nc.compile()
    return nc


def _build_body(nc, tc, x0T_d, wqkv_d, bqkv_d, wo_d, w1_d, b1_d, w2_d,
                brows_d, wlm_d, blm_d, m01_d, outT_d):
    import contextlib
    ctx = contextlib.ExitStack()
    p_const = ctx.enter_context(tc.tile_pool(name="consts", bufs=1))
    p_x = ctx.enter_context(tc.tile_pool(name="x", bufs=1))
    p_w = ctx.enter_context(tc.tile_pool(name="w", bufs=1))
    p_xn = ctx.enter_context(tc.tile_pool(name="xn", bufs=2))
    p_xsq = ctx.enter_context(tc.tile_pool(name="xsq", bufs=1))
    p_rows = ctx.enter_context(tc.tile_pool(name="rows", bufs=1))
    p_bc = ctx.enter_context(tc.tile_pool(name="bc", bufs=1))
    p_qk = ctx.enter_context(tc.tile_pool(name="qk", bufs=2))
    p_v = ctx.enter_context(tc.tile_pool(name="v", bufs=1))
    p_e = ctx.enter_context(tc.tile_pool(name="e", bufs=2))
    p_sm = ctx.enter_context(tc.tile_pool(name="sm", bufs=2))
    p_attc = ctx.enter_context(tc.tile_pool(name="attc", bufs=2))
    p_a = ctx.enter_context(tc.tile_pool(name="a", bufs=3))
    p_out = ctx.enter_context(tc.tile_pool(name="out", bufs=1))
    ps_ap = ctx.enter_context(tc.tile_pool(name="ps_ap", bufs=2, space="PSUM"))
    ps_big = ctx.enter_context(tc.tile_pool(name="ps_big", bufs=3, space="PSUM"))
    ps_fp2 = ctx.enter_context(tc.tile_pool(name="ps_fp2", bufs=1, space="PSUM"))

    # ---- constants ----
    stage = p_const.tile([128, 8], f32, tag="stage")
    onesC = p_const.tile([128, 2], f32r, tag="onesC")      # 1/C for mean matmuls
    nc.vector.memset(stage[:, 0:2], 1.0 / C)
    nc.vector.tensor_copy(onesC[:], stage[:, 0:2])
    onesH = p_const.tile([128, H], f32r, tag="onesH")      # ones col for V_ext
    nc.vector.memset(stage[:, 2:2 + H], 1.0)
    nc.vector.tensor_copy(onesH[:], stage[:, 2:2 + H])
    stage_row = p_const.tile([1, 512], f32, tag="stage_row")
    nc.vector.memset(stage_row, 1.0)
    onesrow = p_const.tile([1, 512], f32r, tag="onesrow")  # moving row for folds
    nc.vector.tensor_copy(onesrow[:], stage_row[:])
    eps_t = p_const.tile([1, 1], f32, tag="eps")
    nc.vector.memset(eps_t, EPS)
    m01 = p_const.tile([128, 512], f32, tag="m01")
    nc.sync.dma_start(out=m01, in_=m01_d)
    blm_t = p_const.tile([V, 1], f32, tag="blm")
    nc.sync.dma_start(out=blm_t, in_=blm_d.rearrange("(v o) -> v o", o=1))
    wlm_t = [p_const.tile([128, V], f32r, tag=f"wlm{kc}", name=f"wlm{kc}")
             for kc in range(KC)]
    for kc in range(KC):
        nc.sync.dma_start(out=wlm_t[kc], in_=wlm_d[kc * 128:(kc + 1) * 128, :])

    # ---- residual stream ----
    x_t = [[p_x.tile([128, 512], f32r, tag=f"x{kc}_{nt}", name=f"x{kc}_{nt}")
            for nt in range(NBP)] for kc in range(KC)]
    for kc in range(KC):
        for nt in range(NBP):
            nc.sync.dma_start(out=x_t[kc][nt],
                              in_=x0T_d[kc * 128:(kc + 1) * 128,
                                        nt * 512:nt * 512 + 512])

    weights = {}

    def load_wqkv(l):
        w = weights.setdefault(l, {})
        w["wqkv"] = [p_w.tile([128, 3 * C], f32r, tag=f"wqkv{kc}",
                              name=f"wqkv{kc}", bufs=2) for kc in range(KC)]
        for kc in range(KC):
            nc.sync.dma_start(out=w["wqkv"][kc],
                              in_=wqkv_d[l, kc * 128:(kc + 1) * 128, :])
        w["bqkv"] = p_w.tile([128, 6], f32, tag="bqkv", name="bqkv", bufs=2)
        nc.sync.dma_start(out=w["bqkv"], in_=bqkv_d[l])

    def load_rest(l):
        w = weights.setdefault(l, {})
        w["wo"] = [p_w.tile([128, C], f32r, tag=f"wo{kc}", name=f"wo{kc}")
                   for kc in range(KC)]
        for kc in range(KC):
            nc.sync.dma_start(out=w["wo"][kc],
                              in_=wo_d[l, kc * 128:(kc + 1) * 128, :])
        w["w1"] = [p_w.tile([128, DFF], f32r, tag=f"w1{kc}", name=f"w1{kc}")
                   for kc in range(KC)]
        for kc in range(KC):
            nc.sync.dma_start(out=w["w1"][kc],
                              in_=w1_d[l, kc * 128:(kc + 1) * 128, :])
        w["b1"] = p_w.tile([128, K12], f32, tag="b1", name="b1")
        nc.sync.dma_start(out=w["b1"], in_=b1_d[l])
        w["w2"] = [p_w.tile([128, C], f32r, tag=f"w2_{kc}", name=f"w2_{kc}")
                   for kc in range(K12)]
        for kc in range(K12):
            nc.sync.dma_start(out=w["w2"][kc],
                              in_=w2_d[l, kc * 128:(kc + 1) * 128, :])
        w["brows"] = p_w.tile([1, 2 * C], f32r, tag="brows", name="brows")
        nc.sync.dma_start(out=w["brows"], in_=brows_d[l])

    def ln_block(nt, tagp):
        """Standardize x_t[:, nt]: returns list of 3 [128,512] f32r tiles."""
        mu_t = ps_ap.tile([2, 512], f32, tag="ap", name="mu_t")
        sq_t = ps_ap.tile([2, 512], f32, tag="ap", name="sq_t")
        for kc in range(KC):
            nc.tensor.matmul(mu_t[0:2, :], onesC[:], x_t[kc][nt][:],
                             start=(kc == 0), stop=(kc == KC - 1))
        for kc in range(KC):
            xsq = p_xsq.tile([128, 512], f32r, tag="xsq", name="xsq")
            nc.gpsimd.tensor_mul(xsq[:], x_t[kc][nt][:], x_t[kc][nt][:])
            nc.tensor.matmul(sq_t[0:2, :], onesC[:], xsq[:],
                             start=(kc == 0), stop=(kc == KC - 1))
        rows = p_rows.tile([1, 3 * 512], f32, tag="lnrows", name="lnrows")
        A, Br, Cr = rows[:, 0:512], rows[:, 512:1024], rows[:, 1024:1536]
        nc.scalar.copy(A, mu_t[0:1, :])
        nc.scalar.copy(Br, sq_t[0:1, :])
        nc.vector.tensor_mul(Cr, A, A)
        nc.vector.tensor_tensor(out=Br, in0=Br, in1=Cr, op=ALU.subtract)
        nc.scalar.activation(Br, Br, AF.Sqrt, bias=eps_t[:], scale=1.0)
        nc.vector.reciprocal_approx_fast(out=Br, in_=Br)      # rs
        nc.vector.tensor_mul(Cr, A, Br)                       # mr = mu*rs
        rs_b = p_bc.tile([128, 512], f32, tag="rs_b", name="rs_b")
        mr_b = p_bc.tile([128, 512], f32, tag="mr_b", name="mr_b")
        nc.gpsimd.partition_broadcast(rs_b[:], Br)
        nc.gpsimd.partition_broadcast(mr_b[:], Cr)
        outs = []
        for kc in range(KC):
            xs = p_xsq.tile([128, 512], f32, tag="xs", name="xs")
            nc.gpsimd.tensor_mul(xs[:], x_t[kc][nt][:], rs_b[:])
            o = p_xn.tile([128, 512], f32r, tag=f"{tagp}{kc}", name=f"{tagp}{kc}")
            nc.vector.tensor_tensor(out=o[:], in0=xs[:], in1=mr_b[:],
                                    op=ALU.subtract)
            outs.append(o)
        return outs

    state = {}

    def emit_A(it):
        l, bp = divmod(it, NBP)
        w = weights[l]
        xn = ln_block(bp, "xn")
        qk = []
        for oc in range(6):
            qp = ps_big.tile([128, 512], f32, tag="big", name="qp")
            for kc in range(KC):
                nc.tensor.matmul(qp[:], w["wqkv"][kc][:, oc * 128:oc * 128 + 128],
                                 xn[kc][:], start=(kc == 0), stop=(kc == KC - 1))
            qt = p_qk.tile([128, 512], f32r, tag=f"qk{oc}", name=f"qk{oc}")
            nc.scalar.activation(qt[:], qp[:], AF.Identity,
                                 bias=w["bqkv"][:, oc:oc + 1], scale=1.0)
            qk.append(qt)
        vext = []
        for bi in range(2):
            vx = p_v.tile([128, 2 * H * (HS + 1)], f32r, tag=f"vext{bi}",
                          name=f"vext{bi}")
            vxr = vx.rearrange("p (j h e) -> p j h e", j=2, h=H)
            for j in range(2):
                vp = ps_big.tile([128, C], f32, tag="big", name="vp")
                tc0 = bi * 256 + j * 128
                for kc in range(KC):
                    nc.tensor.matmul(vp[:], xn[kc][:, tc0:tc0 + 128],
                                     w["wqkv"][kc][:, 2 * C:3 * C],
                                     start=(kc == 0), stop=(kc == KC - 1))
                nc.vector.tensor_copy(vxr[:, j, :, 0:HS],
                                      vp[:].rearrange("p (h d) -> p h d", h=H))
                nc.gpsimd.tensor_copy(out=vxr[:, j, :, HS:HS + 1], in_=onesH[:])
            vext.append(vx)
        state[it] = {"xn": xn, "qk": qk, "vext": vext}

    def emit_ATT(it):
        st = state[it]
        qk, vext = st["qk"], st["vext"]
        attc = [p_attc.tile([128, 512], f32r, tag=f"attc{kc}", name=f"attc{kc}")
                for kc in range(KC)]
        for h in range(H):
            qch, kch = h // 2, 3 + h // 2
            qrow = (h % 2) * 64
            ap_ = ps_ap.tile([HS + 1, 512], f32, tag="ap", name="ap_")
            for bi in range(2):
                q0 = bi * 256
                sp = ps_big.tile([128, 512], f32, tag="big", name="sp")
                qs = qk[qch][qrow:qrow + 64, q0:q0 + 256]
                nc.tensor.matmul(sp[:, 0:256],
                                 qk[kch][qrow:qrow + 64, q0:q0 + 128],
                                 qs, start=True, stop=True)
                nc.tensor.matmul(sp[:, 256:512],
                                 qk[kch][qrow:qrow + 64, q0 + 128:q0 + 256],
                                 qs, start=True, stop=True)
                e_t = p_e.tile([128, 512], f32, tag="e_t", name="e_t")
                nc.scalar.activation(e_t[:], sp[:], AF.Exp, bias=0.0,
                                     scale=SCALE)
                e_m = p_e.tile([128, 512], f32r, tag="e_m", name="e_m")
                nc.gpsimd.tensor_mul(e_m[:], e_t[:], m01[:])
                vxr = vext[bi].rearrange("p (j h e) -> p j h e", j=2, h=H)
                nc.tensor.matmul(ap_[:, q0:q0 + 256], vxr[:, 0, h, :],
                                 e_m[:, 0:256], start=True, stop=False)
                nc.tensor.matmul(ap_[:, q0:q0 + 256], vxr[:, 1, h, :],
                                 e_m[:, 256:512], start=False, stop=True)
            srow = p_sm.tile([1, 512], f32, tag="srow", name="srow", bufs=1)
            nc.scalar.copy(srow[:], ap_[HS:HS + 1, :])
            rec = p_sm.tile([1, 512], f32, tag="rec", name="rec", bufs=1)
            nc.vector.reciprocal_approx_fast(out=rec[:], in_=srow[:])
            r_b = p_sm.tile([64, 512], f32, tag="r_b", name="r_b")
            nc.gpsimd.partition_broadcast(r_b[:], rec[:])
            nc.vector.tensor_mul(attc[qch][qrow:qrow + 64, :],
                                 ap_[0:HS, :], r_b[:])
        state[it]["attc"] = attc
        del state[it]["qk"], state[it]["vext"]

    def emit_WoLN2(it):
        l, bp = divmod(it, NBP)
        w = weights[l]
        attc = state[it]["attc"]
        for oc in range(KC):
            wp = ps_big.tile([128, 512], f32, tag="big", name="wp")
            nc.tensor.matmul(wp[:], w["brows"][0:1, oc * 128:oc * 128 + 128],
                             onesrow[:], start=True, stop=False)
            for kc in range(KC):
                nc.tensor.matmul(wp[:], w["wo"][kc][:, oc * 128:oc * 128 + 128],
                                 attc[kc][:], start=False, stop=(kc == KC - 1))
            nc.vector.tensor_tensor(out=x_t[oc][bp][:], in0=wp[:],
                                    in1=x_t[oc][bp][:], op=ALU.add)
        state[it]["h2n"] = ln_block(bp, "h2n")
        del state[it]["attc"]

    def emit_FFN(it):
        l, bp = divmod(it, NBP)
        w = weights[l]
        h2n = state[it]["h2n"]
        fp2 = [ps_fp2.tile([128, 512], f32, tag=f"fp2_{oc}", name=f"fp2_{oc}")
               for oc in range(KC)]
        for oc in range(KC):
            nc.tensor.matmul(fp2[oc][:],
                             w["brows"][0:1, C + oc * 128:C + oc * 128 + 128],
                             onesrow[:], start=True, stop=False)
        for kc12 in range(K12):
            fp1 = ps_big.tile([128, 512], f32, tag="big", name="fp1")
            for kc in range(KC):
                nc.tensor.matmul(fp1[:],
                                 w["w1"][kc][:, kc12 * 128:kc12 * 128 + 128],
                                 h2n[kc][:], start=(kc == 0), stop=(kc == KC - 1))
            a = p_a.tile([128, 512], f32r, tag="a", name="a")
            nc.scalar.activation(a[:], fp1[:], AF.Relu,
                                 bias=w["b1"][:, kc12:kc12 + 1], scale=1.0)
            for oc in range(KC):
                nc.tensor.matmul(fp2[oc][:],
                                 w["w2"][kc12][:, oc * 128:oc * 128 + 128],
                                 a[:], start=False, stop=(kc12 == K12 - 1))
        for oc in range(KC):
            nc.vector.tensor_tensor(out=x_t[oc][bp][:], in0=fp2[oc][:],
                                    in1=x_t[oc][bp][:], op=ALU.add)
        del state[it]

    # ---- pipelined emission ----
    NITER = N_LAYERS * NBP
    load_wqkv(0)
    load_rest(0)
    for it in range(NITER + 1):
        if it < NITER:
            l, bp = divmod(it, NBP)
            if bp == 2 and l + 1 < N_LAYERS:
                load_wqkv(l + 1)
            emit_A(it)
        if it >= 1:
            emit_WoLN2(it - 1)
        if it < NITER:
            emit_ATT(it)
        if it >= 1:
            emit_FFN(it - 1)
            pl, pbp = divmod(it - 1, NBP)
            if pbp == NBP - 1 and pl + 1 < N_LAYERS:
                load_rest(pl + 1)

    # ---- final LN + LM head ----
    for nt in range(NBP):
        xf = ln_block(nt, "xn")
        lp = ps_big.tile([V, 512], f32, tag="big", name="lp")
        for kc in range(KC):
            nc.tensor.matmul(lp[:], wlm_t[kc][:], xf[kc][:],
                             start=(kc == 0), stop=(kc == KC - 1))
        osb = p_out.tile([V, 512], f32, tag="osb", name="osb")
        nc.scalar.activation(osb[:], lp[:], AF.Identity, bias=blm_t[:],
                             scale=1.0)
        nc.sync.dma_start(out=outT_d[:, nt * 512:nt * 512 + 512], in_=osb[:])

    ctx.close()


def _host_prep(inputs):
    """Fold LN affine params into weights; build per-core input maps."""
    f = lambda k: np.asarray(inputs[k], dtype=np.float32)
    idx = np.asarray(inputs["idx"]).astype(np.int64)
    tok_emb, pos_emb = f("tok_emb"), f("pos_emb")
    Wq, Wk, Wv, Wo = f("Wq"), f("Wk"), f("Wv"), f("Wo")
    bo, W1, b1, W2, b2 = f("bo"), f("W1"), f("b1"), f("W2"), f("b2")
    ln1_g, ln1_b = f("ln1_g"), f("ln1_b")
    ln2_g, ln2_b = f("ln2_g"), f("ln2_b")
    lnf_g, lnf_b = f("lnf_g"), f("lnf_b")
    Wlm, blm = f("Wlm"), f("blm")

    # [L,H,C,HS] -> [L,C,H*HS]
    Wq_all = np.transpose(Wq, (0, 2, 1, 3)).reshape(L, C, C)
    Wk_all = np.transpose(Wk, (0, 2, 1, 3)).reshape(L, C, C)
    Wv_all = np.transpose(Wv, (0, 2, 1, 3)).reshape(L, C, C)

    g1 = ln1_g[:, :, None]
    wqkv = np.concatenate([g1 * Wq_all, g1 * Wk_all, g1 * Wv_all], axis=2)
    bq = np.einsum("lc,lcd->ld", ln1_b, Wq_all)
    bk = np.einsum("lc,lcd->ld", ln1_b, Wk_all)
    bv = np.einsum("lc,lcd->ld", ln1_b, Wv_all)
    bo2 = bo + np.einsum("ld,ldc->lc", bv, Wo)       # v-bias folds through Wo
    w1f = ln2_g[:, :, None] * W1
    b1f = b1 + np.einsum("lc,lcd->ld", ln2_b, W1)
    wlmf = lnf_g[:, None] * Wlm
    blmf = blm + lnf_b @ Wlm

    bqkv = np.concatenate([bq, bk], axis=1)          # [L, 768]
    bqkv_cols = np.ascontiguousarray(
        bqkv.reshape(L, 6, 128).transpose(0, 2, 1))  # [L,128,6]
    b1_cols = np.ascontiguousarray(
        b1f.reshape(L, K12, 128).transpose(0, 2, 1))  # [L,128,12]
    brows = np.ascontiguousarray(
        np.concatenate([bo2, b2], axis=1)[:, None, :])  # [L,1,2C]

    # multiplicative causal mask, key-major: cols = (key_block, q)
    p = np.arange(128)[:, None]
    q = np.arange(256)[None, :]
    m0 = (p <= q).astype(np.float32)          # keys 0..127
    m1 = (p + 128 <= q).astype(np.float32)    # keys 128..255
    m01 = np.concatenate([m0, m1], axis=1)    # [128, 512]

    x0 = tok_emb[idx] + pos_emb[None]                # [B,T,C] f32
    in_maps = []
    for c in range(N_CORES):
        x0c = x0[c * BPC:(c + 1) * BPC].reshape(NTOK, C)
        in_maps.append({
            "x0T": np.ascontiguousarray(x0c.T),
            "wqkv": np.ascontiguousarray(wqkv),
            "bqkv": bqkv_cols,
            "wo": np.ascontiguousarray(Wo),
            "w1": np.ascontiguousarray(w1f),
            "b1": b1_cols,
            "w2": np.ascontiguousarray(W2),
            "brows": brows,
            "wlm": np.ascontiguousarray(wlmf),
            "blm": np.ascontiguousarray(blmf),
            "m01": m01,
        })
    return in_maps


def _run(inputs, trace=False):
    if "nc" not in _cache:
        _cache["nc"] = _build_nc()
    nc = _cache["nc"]
    in_maps = _host_prep(inputs)
    res = run_bass_kernel_spmd(nc, in_maps, core_ids=list(range(N_CORES)),
                               trace=trace)
    outs = []
    for c in range(N_CORES):
        outT = res.results[c]["outT"]                 # [V, NTOK]
        outs.append(outT.T.reshape(BPC, T, V))
    logits = np.concatenate(outs, axis=0).astype(np.float32)
    return logits, res


def kernel(**inputs) -> np.ndarray:
    logits, _ = _run(inputs, trace=False)
    return logits
